# revision 27
# baseline (speedup 1.0000x reference)
"""Binary complex conv (BC conv) on 8 TRN2 NeuronCores.

Reference computation:
    xb = sign(x + 1e-6)                      # (16, 256, 112, 112)
    wr = sign(weight_real + 1e-6)            # (128, 128, 3, 3)
    wi = sign(weight_imag + 1e-6)
    kernel = [[wr, -wi], [wi, wr]]           # (256, 256, 3, 3)
    out = conv2d(xb, kernel, pad=1) + bias   # (16, 256, 112, 112)

Strategy: pure data-parallel over batch (2 images per core); everything
else on-device, numerically exact (all matmul operands are small dyadic
rationals -> exact in fp8; PSUM accumulates fp32).

Tricks on top of the direct conv:
 * Karatsuba for the complex structure: A = xr*wr, B = xi*wi,
   C = (xr+xi)*(wr+wi); out_real = A-B, out_imag = C-A-B.
   3 convs of 128 input channels instead of 4.
 * fp8 DoubleRow: binarized frames with row stride 114; conv taps in
   raster order have flat offsets [-115,-114,-113,-1,0,1,113,114,115];
   consecutive taps pair into DoubleRow matmuls (contraction 256).
 * inputs binarized to {0,1} (pads 0.5) by a cheap VectorE compare
   instead of a ScalarE Sign; the identity sign(x) = 2*step(x) - 1
   turns each conv into 2*conv_hat - K[co], with the per-out-channel
   K = sum(w) computed once by 10 tiny matmuls and folded into the
   PSUM-evacuation biases.
 * 4-level moving APs: matmul free dim is exactly 4 rows x 112 cols
   (448), no pad lanes; PSUM evacuations are contiguous.
 * output staged in 4-tile blocks so each DMA descriptor line is
   16 output rows (7168B) -- the 16 DMA engines run near their
   per-engine rate cap, which is the true limiter at this pace.
"""

import numpy as np

import concourse.bass as bass
import concourse.tile as tile
from concourse import mybir
from concourse.bass_utils import run_bass_kernel_spmd

N_CORES = 8
B = 16
CPB = 128          # channels per block (partition dim)
H = W = 112
HW = H * W
RS = 114           # frame row stride
FRO = 115          # frame rows: rows 1..114 used (1=top pad, 114=bottom pad)
IMGS = 2
TROWS = 4          # output rows per matmul tile
NT = TROWS * W     # matmul free dim (448)
QUAD = 4           # output tiles aggregated per store DMA
EPS = 1e-6

F32 = mybir.dt.float32
FP8 = mybir.dt.float8e4
AF = mybir.ActivationFunctionType
DRM = mybir.MatmulPerfMode.DoubleRow
ALU = mybir.AluOpType

# tap flat offsets in raster order; pairs (0,1) (2,3) (4,5) (6,7), single 8
TAP_OFF = [dy * RS + dx for dy in (-1, 0, 1) for dx in (-1, 0, 1)]


def _split_multiwait(nc):
    """Walrus in this container rejects >1 semaphore wait per instruction
    ("Too many sync wait commands"); hoist extra waits onto preceding nops
    on the same engine."""
    import bass_rust

    for f in nc.m.functions:
        for bb in f.blocks:
            new_insts = []
            for inst in bb.instructions:
                si = inst.sync_info
                waits = list(si.on_wait) if si is not None and si.on_wait else []
                if len(waits) > 1:
                    for w in waits[:-1]:
                        nop = mybir.InstNoOp(
                            name=nc.get_next_instruction_name(),
                            engine=inst.engine,
                            ins=[],
                            outs=[],
                        )
                        nop.sync_info = bass_rust.SyncInfo(on_wait=[w], on_update=[])
                        new_insts.append(nop)
                    si.on_wait = [waits[-1]]
                    inst.sync_info = si
                new_insts.append(inst)
            bb.instructions = new_insts


def build_nc():
    nc = bass.Bass()

    x_ext = nc.declare_dram_parameter("x", [IMGS, 2 * CPB, H, W], F32, isOutput=False)
    wr_ext = nc.declare_dram_parameter("wrT", [CPB, 9 * CPB], F32, isOutput=False)
    wi_ext = nc.declare_dram_parameter("wiT", [CPB, 9 * CPB], F32, isOutput=False)
    bias_ext = nc.declare_dram_parameter("bias2", [CPB, 2], F32, isOutput=False)
    out_ext = nc.declare_dram_parameter("out", [IMGS, 2 * CPB, H, W], F32, isOutput=True)

    x2 = x_ext.rearrange("b c h w -> (b c) (h w)")       # [512, 12544]
    out2 = out_ext.rearrange("b c h w -> (b c) (h w)")

    with tile.TileContext(nc) as tc:
        with (
            tc.tile_pool(name="wstage", bufs=2) as wstage_pool,
            tc.tile_pool(name="wbin", bufs=1) as wbin_pool,
            tc.tile_pool(name="biasp", bufs=1) as bias_pool,
            tc.tile_pool(name="xq", bufs=1) as xq_pool,
            tc.tile_pool(name="stage", bufs=3) as stage_pool,
            tc.tile_pool(name="tmp", bufs=9) as tmp_pool,
            tc.tile_pool(name="outsb", bufs=2) as out_pool,
            tc.tile_pool(name="psum", bufs=8, space="PSUM") as psum_pool,
        ):
            # per-partition scalar constant for weight-sign activation bias
            eps_pos = bias_pool.tile([CPB, 1], F32, tag="epsp")
            nc.gpsimd.memset(eps_pos[:], EPS)
            ones2 = bias_pool.tile([CPB, 2], FP8, tag="ones2")
            nc.gpsimd.memset(ones2[:], 1.0)

            # HAM warmup: dummy matmuls on junk data with no dependencies so
            # the PE clock-gate reaches 8/8 before the first real matmul;
            # sized to end right when the first binarized strip is ready
            junk = bias_pool.tile([CPB, 512], FP8, tag="junk")
            nc.gpsimd.memset(junk[:, 0:1], 1.0)
            jps = psum_pool.tile([CPB, 512], F32, tag="ps", name="jps")
            for _ in range(10):
                nc.tensor.matmul(jps[:], junk[:, :CPB], junk[:], start=True,
                                 stop=True)
            for _ in range(4):
                nc.tensor.matmul(jps[:, :256], junk[:, :CPB], junk[:, :256],
                                 start=True, stop=True)

            # ---- weights first (2304B packets), then input strips ----
            wr_f32 = wstage_pool.tile([CPB, 9 * CPB], F32, tag="wstage")
            nc.sync.dma_start(wr_f32[:, :576], wr_ext[:, :576])
            nc.sync.dma_start(wr_f32[:, 576:], wr_ext[:, 576:])
            wi_f32 = wstage_pool.tile([CPB, 9 * CPB], F32, tag="wstage")
            nc.sync.dma_start(wi_f32[:, :576], wi_ext[:, :576])
            nc.sync.dma_start(wi_f32[:, 576:], wi_ext[:, 576:])

            def load_strip(img, r0, nr):
                st = stage_pool.tile([CPB, 2, 14, W], F32, tag="stage")
                src = bass.AP(
                    x2.tensor,
                    x2.offset + img * 2 * CPB * HW + r0 * W,
                    [[HW, CPB], [CPB * HW, 2], [1, nr * W]],
                )
                nc.sync.dma_start(st[:, :, :nr, :], src)
                return st

            st0a = load_strip(0, 0, 5)
            bias_sb = bias_pool.tile([CPB, 2], F32)
            nc.sync.dma_start(bias_sb[:], bias_ext[:])

            # binarized fp8 weights [ci, tap, co]; wq_s = wq_r + wq_i
            wq_r = wbin_pool.tile([CPB, 9, CPB], FP8, tag="wqr")
            wq_i = wbin_pool.tile([CPB, 9, CPB], FP8, tag="wqi")
            wq_s = wbin_pool.tile([CPB, 9, CPB], FP8, tag="wqs")
            wr_v = wr_f32[:].rearrange("p (t c) -> p t c", c=CPB)
            wi_v = wi_f32[:].rearrange("p (t c) -> p t c", c=CPB)
            nc.scalar.activation(wq_r[:], wr_v, AF.Sign, bias=eps_pos[:], scale=1.0)
            nc.scalar.activation(wq_i[:], wi_v, AF.Sign, bias=eps_pos[:], scale=1.0)

            wq8h = wbin_pool.tile([CPB, 3, 2, CPB], FP8, tag="wq8h")

            # ---- K constants: K[co] = sum_{ci,tap} w  (KC = KA + KB) ----
            psK = psum_pool.tile([CPB, 2], F32, tag="ps", name="psK")
            for k, wq in ((0, wq_r), (1, wq_i)):
                for p in range(4):
                    rhs = bass.AP(ones2[:].tensor, ones2[:].offset,
                                  [[ones2[:].ap[0][0], CPB], [1, 2], [1, 1]])
                    nc.tensor.matmul(psK[:, k:k + 1], wq[:, 2 * p:2 * p + 2, :],
                                     rhs, start=(p == 0), stop=False,
                                     perf_mode=DRM)
                nc.tensor.matmul(psK[:, k:k + 1], wq[:, 8, :], ones2[:, 0:1],
                                 start=False, stop=True)

            # ---- persistent binarized {0,1} fp8 frames ----
            # frame row = input row + 2; frame rows 1 / 114 are the pad rows,
            # cols 0 / 113 the pad cols; pads hold 0.5 (r,i) / 1.0 (s) so
            # that x = 2*xhat - 1 (resp. xs = 2*shat - 2) is 0 there.
            xq = [xq_pool.tile([CPB, 3, FRO, RS], FP8, tag=f"xq{i}", name=f"xq{i}")
                  for i in range(IMGS)]
            for i in range(IMGS):
                eng = nc.vector if i == 0 else nc.gpsimd
                for (ks, val) in ((slice(0, 2), 0.5), (slice(2, 3), 1.0)):
                    eng.memset(xq[i][:, ks, 1:2, :], val)       # pad row top
                    eng.memset(xq[i][:, ks, 114:115, :], val)   # pad row bottom
                    eng.memset(xq[i][:, ks, 1:115, 0:1], val)   # pad col left
                    eng.memset(xq[i][:, ks, 1:115, 113:114], val)  # pad col right

            flat = [xq[i][:].rearrange("p k r c -> p (k r c)") for i in range(IMGS)]
            KOFS = FRO * RS

            def binarize_strip(img, r0, nr, st):
                rows = slice(r0 + 2, r0 + 2 + nr)
                nc.vector.tensor_scalar(
                    xq[img][:, 0:2, rows, 1:113], st[:, :, :nr, :],
                    -EPS, None, op0=ALU.is_ge,
                )
                nc.gpsimd.tensor_tensor(
                    xq[img][:, 2, rows, 1:113],
                    xq[img][:, 0, rows, 1:113],
                    xq[img][:, 1, rows, 1:113],
                    op=ALU.add,
                )

            binarize_strip(0, 0, 5, st0a)
            # wq_s add waits on the wq_i Sign; emit it after the first strip's
            # binarize so it can't head-of-line-block that in the vector FIFO
            nc.vector.tensor_tensor(wq_s[:], wq_r[:], wq_i[:], op=ALU.add)
            # half-scaled tap-8 weights: [w8/2; w8/2] pairs let the 9th tap
            # run as a stride-0 DoubleRow matmul, so every matmul in the
            # kernel is DR -- no FWL<->DoubleRow mode switches on the PE
            for k, wq in ((0, wq_r), (1, wq_i), (2, wq_s)):
                for j in range(2):
                    nc.vector.tensor_scalar(wq8h[:, k, j, :], wq[:, 8, :],
                                            0.5, None, op0=ALU.mult)

            # beta1 = (br-bi)/2 + KB   (An2 = 2*Ahat + beta1)
            # beta2 = (br+bi)/2 - KA   (Bn0 = -2*Bhat + beta2)
            kvec = bias_pool.tile([CPB, 2], F32, tag="kvec")
            nc.vector.tensor_copy(kvec[:], psK[:])
            beta1 = bias_pool.tile([CPB, 1], F32, tag="b1")
            beta2 = bias_pool.tile([CPB, 1], F32, tag="b2")
            hdif = bias_pool.tile([CPB, 1], F32, tag="hd")
            hsum = bias_pool.tile([CPB, 1], F32, tag="hs")
            nc.vector.tensor_sub(hdif[:], bias_sb[:, 0:1], bias_sb[:, 1:2])
            nc.vector.tensor_add(hsum[:], bias_sb[:, 0:1], bias_sb[:, 1:2])
            nc.vector.scalar_tensor_tensor(beta1[:], hdif[:], 0.5, kvec[:, 1:2],
                                           op0=ALU.mult, op1=ALU.add)
            nc.vector.scalar_tensor_tensor(beta2[:], hsum[:], 0.5, kvec[:, 0:1],
                                           op0=ALU.mult, op1=ALU.subtract)

            st0b = load_strip(0, 5, 9)
            binarize_strip(0, 5, 9, st0b)

            # ---- conv tiles ----
            quads = {}

            def conv_tile(img, t, last_quad=False):
                xf = flat[img]
                part = [list(xf.ap)[0][0], CPB]
                pk = {}
                for ki, (kind, w3) in enumerate((("r", wq_r), ("i", wq_i),
                                                 ("s", wq_s))):
                    base = KOFS * ki + (4 * t + 2) * RS + 1
                    ps = psum_pool.tile([CPB, NT], F32, tag="ps",
                                        name=f"ps_{kind}{img}_{t}")
                    pk[kind] = ps
                    for p in range(4):
                        o0, o1 = TAP_OFF[2 * p], TAP_OFF[2 * p + 1]
                        rhs = bass.AP(
                            xf.tensor, xf.offset + base + o0,
                            [part, [o1 - o0, 2], [RS, TROWS], [1, W]],
                        )
                        nc.tensor.matmul(
                            ps[:], w3[:, 2 * p:2 * p + 2, :], rhs,
                            start=(p == 0), stop=False, perf_mode=DRM,
                        )
                    rhs8 = bass.AP(
                        xf.tensor, xf.offset + base + TAP_OFF[8],
                        [part, [0, 2], [RS, TROWS], [1, W]],
                    )
                    nc.tensor.matmul(ps[:], wq8h[:, ki], rhs8,
                                     start=False, stop=True, perf_mode=DRM)

                q, qi = t // QUAD, t % QUAD
                key = (img, q)
                if key not in quads:
                    quads[key] = out_pool.tile([CPB, 2, QUAD, NT], F32,
                                               tag="osb", name=f"osb{img}_{q}")
                ob = quads[key]

                An2 = tmp_pool.tile([CPB, NT], F32, tag="An", name=f"An{t}")
                nc.scalar.activation(An2[:], pk["r"][:], AF.Identity,
                                     bias=beta1[:], scale=2.0)
                Bn0 = tmp_pool.tile([CPB, NT], F32, tag="Bn", name=f"Bn{t}")
                nc.scalar.activation(Bn0[:], pk["i"][:], AF.Identity,
                                     bias=beta2[:], scale=-2.0)
                t5 = tmp_pool.tile([CPB, NT], F32, tag="t5", name=f"t5{t}")
                nc.vector.scalar_tensor_tensor(t5[:], pk["s"][:], 2.0, An2[:],
                                               op0=ALU.mult, op1=ALU.subtract)
                nc.gpsimd.tensor_tensor(ob[:, 0, qi], An2[:], Bn0[:], op=ALU.add)
                nc.vector.tensor_tensor(ob[:, 1, qi], t5[:], Bn0[:], op=ALU.add)

                if last_quad:
                    # final tiles: store per tile so the drain tail is short
                    dst = bass.AP(
                        out2.tensor,
                        out2.offset + img * 2 * CPB * HW + 4 * t * W,
                        [[HW, CPB], [CPB * HW, 2], [1, NT]],
                    )
                    nc.sync.dma_start(dst, ob[:, :, qi])
                elif qi == QUAD - 1:
                    dst = bass.AP(
                        out2.tensor,
                        out2.offset + img * 2 * CPB * HW + 4 * QUAD * q * W,
                        [[HW, CPB], [CPB * HW, 2], [1, QUAD * NT]],
                    )
                    nc.sync.dma_start(dst, ob[:])

            # ---- strip schedule ----
            # strip s of an image covers input rows 14s..14s+13; the first
            # tile needing it is ceil((14s - 4) / 4).  The DMA takes ~9us
            # from issue (DGE latency + queueing behind store bursts), so
            # issue it 8 tiles ahead; binarize 2 tiles ahead so the wait
            # never poisons the vector FIFO.
            first_need = [0, 3, 6, 10, 13, 17, 20, 24]
            dma_at = {}
            ts_at = {}
            pending = {}
            for img in range(IMGS):
                for s in range(8):
                    if img == 0 and s == 0:
                        continue
                    g = 28 * img + first_need[s]
                    dma_at.setdefault(max(g - 8, -1), []).append((img, s))
                    ts_at.setdefault(max(g - 2, -1), []).append((img, s))

            def do_sched(gt):
                for (img, s) in dma_at.get(gt, ()):
                    pending[(img, s)] = load_strip(img, 14 * s, 14)
                for (img, s) in ts_at.get(gt, ()):
                    binarize_strip(img, 14 * s, 14, pending.pop((img, s)))

            do_sched(-1)
            for gt in range(56):
                img, t = gt // 28, gt % 28
                conv_tile(img, t, last_quad=(gt >= 48))
                do_sched(gt)

    _split_multiwait(nc)
    return nc


def _prep(x, weight_real, weight_imag, bias):
    x = np.ascontiguousarray(np.asarray(x, dtype=np.float32))
    wr = np.asarray(weight_real, dtype=np.float32)
    wi = np.asarray(weight_imag, dtype=np.float32)
    bias = np.asarray(bias, dtype=np.float32)
    wrT = np.ascontiguousarray(wr.transpose(1, 2, 3, 0).reshape(CPB, 9 * CPB))
    wiT = np.ascontiguousarray(wi.transpose(1, 2, 3, 0).reshape(CPB, 9 * CPB))
    bias2 = np.ascontiguousarray(bias.reshape(2, CPB).T)
    return [
        {"x": x[IMGS * c:IMGS * (c + 1)], "wrT": wrT, "wiT": wiT, "bias2": bias2}
        for c in range(N_CORES)
    ]


def kernel(x, weight_real, weight_imag, bias):
    in_maps = _prep(x, weight_real, weight_imag, bias)
    nc = build_nc()
    res = run_bass_kernel_spmd(nc, in_maps, core_ids=list(range(N_CORES)))
    return np.concatenate([res.results[i]["out"] for i in range(N_CORES)], axis=0)


def run_traced(x, weight_real, weight_imag, bias, **trace_kwargs):
    """test.py entry: same as kernel() but with neuron-profile tracing."""
    in_maps = _prep(x, weight_real, weight_imag, bias)
    nc = build_nc()
    res = run_bass_kernel_spmd(
        nc, in_maps, core_ids=list(range(N_CORES)), trace=True, **trace_kwargs
    )
    out = np.concatenate([res.results[i]["out"] for i in range(N_CORES)], axis=0)
    return out, res


# revision 29
# speedup vs baseline: 1.2419x; 1.2419x over previous
"""Binary complex conv (BC conv) on 8 TRN2 NeuronCores.

Reference computation:
    xb = sign(x + 1e-6)                      # (16, 256, 112, 112)
    wr = sign(weight_real + 1e-6)            # (128, 128, 3, 3)
    wi = sign(weight_imag + 1e-6)
    kernel = [[wr, -wi], [wi, wr]]           # (256, 256, 3, 3)
    out = conv2d(xb, kernel, pad=1) + bias   # (16, 256, 112, 112)

Strategy: pure data-parallel over batch (2 images per core); everything
else on-device, numerically exact (all matmul operands are small dyadic
rationals -> exact in fp8; PSUM accumulates fp32).

Tricks on top of the direct conv:
 * Karatsuba for the complex structure: A = xr*wr, B = xi*wi,
   C = (xr+xi)*(wr+wi); out_real = A-B, out_imag = C-A-B.
   3 convs of 128 input channels instead of 4.
 * fp8 DoubleRow: binarized frames with row stride 114; conv taps in
   raster order have flat offsets [-115,-114,-113,-1,0,1,113,114,115];
   consecutive taps pair into DoubleRow matmuls (contraction 256).
 * inputs binarized to {0,1} (pads 0.5) by a cheap VectorE compare
   instead of a ScalarE Sign; the identity sign(x) = 2*step(x) - 1
   turns each conv into 2*conv_hat - K[co], with the per-out-channel
   K = sum(w) computed once by 10 tiny matmuls and folded into the
   PSUM-evacuation biases.
 * 4-level moving APs: matmul free dim is exactly 4 rows x 112 cols
   (448), no pad lanes; PSUM evacuations are contiguous.
 * output staged in 4-tile blocks so each DMA descriptor line is
   16 output rows (7168B) -- the 16 DMA engines run near their
   per-engine rate cap, which is the true limiter at this pace.
"""

import numpy as np

import concourse.bass as bass
import concourse.tile as tile
from concourse import mybir
from concourse.bass_utils import run_bass_kernel_spmd

N_CORES = 8
B = 16
CPB = 128          # channels per block (partition dim)
H = W = 112
HW = H * W
RS = 114           # frame row stride
FRO = 116          # frame rows: 1..114 used + row 115 scratch (zero-row tap)
IMGS = 2
TROWS = 4          # output rows per matmul tile
NT = TROWS * W     # matmul free dim (448)
QUAD = 2           # output tiles aggregated per store DMA
EPS = 1e-6

F32 = mybir.dt.float32
FP8 = mybir.dt.float8e4
AF = mybir.ActivationFunctionType
DRM = mybir.MatmulPerfMode.DoubleRow
ALU = mybir.AluOpType

# tap flat offsets in raster order; pairs (0,1) (2,3) (4,5) (6,7), single 8
TAP_OFF = [dy * RS + dx for dy in (-1, 0, 1) for dx in (-1, 0, 1)]


def _split_multiwait(nc):
    """Walrus in this container rejects >1 semaphore wait per instruction
    ("Too many sync wait commands"); hoist extra waits onto preceding nops
    on the same engine."""
    import bass_rust

    for f in nc.m.functions:
        for bb in f.blocks:
            new_insts = []
            for inst in bb.instructions:
                si = inst.sync_info
                waits = list(si.on_wait) if si is not None and si.on_wait else []
                if len(waits) > 1:
                    for w in waits[:-1]:
                        nop = mybir.InstNoOp(
                            name=nc.get_next_instruction_name(),
                            engine=inst.engine,
                            ins=[],
                            outs=[],
                        )
                        nop.sync_info = bass_rust.SyncInfo(on_wait=[w], on_update=[])
                        new_insts.append(nop)
                    si.on_wait = [waits[-1]]
                    inst.sync_info = si
                new_insts.append(inst)
            bb.instructions = new_insts


def build_nc():
    nc = bass.Bass()

    x_ext = nc.declare_dram_parameter("x", [IMGS, 2 * CPB, H, W], F32, isOutput=False)
    wr_ext = nc.declare_dram_parameter("wrT", [CPB, 9 * CPB], F32, isOutput=False)
    wi_ext = nc.declare_dram_parameter("wiT", [CPB, 9 * CPB], F32, isOutput=False)
    bias_ext = nc.declare_dram_parameter("bias2", [CPB, 2], F32, isOutput=False)
    out_ext = nc.declare_dram_parameter("out", [IMGS, 2 * CPB, H, W], F32, isOutput=True)

    x2 = x_ext.rearrange("b c h w -> (b c) (h w)")       # [512, 12544]
    out2 = out_ext.rearrange("b c h w -> (b c) (h w)")

    with tile.TileContext(nc) as tc:
        with (
            tc.tile_pool(name="wstage", bufs=2) as wstage_pool,
            tc.tile_pool(name="wbin", bufs=1) as wbin_pool,
            tc.tile_pool(name="biasp", bufs=1) as bias_pool,
            tc.tile_pool(name="xq", bufs=1) as xq_pool,
            tc.tile_pool(name="stage", bufs=4) as stage_pool,
            tc.tile_pool(name="tmp", bufs=9) as tmp_pool,
            tc.tile_pool(name="outsb", bufs=2) as out_pool,
            tc.tile_pool(name="psum", bufs=8, space="PSUM") as psum_pool,
        ):
            # per-partition scalar constant for weight-sign activation bias
            eps_pos = bias_pool.tile([CPB, 1], F32, tag="epsp")
            nc.gpsimd.memset(eps_pos[:], EPS)
            ones2 = bias_pool.tile([CPB, 2], FP8, tag="ones2")
            nc.gpsimd.memset(ones2[:], 1.0)

            # HAM warmup: dummy matmuls on junk data with no dependencies so
            # the PE clock-gate reaches 8/8 before the first real matmul;
            # sized to end right when the first binarized strip is ready
            junk = bias_pool.tile([CPB, 512], FP8, tag="junk")
            nc.gpsimd.memset(junk[:, 0:1], 1.0)
            jps = psum_pool.tile([CPB, 512], F32, tag="ps", name="jps")
            for _ in range(10):
                nc.tensor.matmul(jps[:], junk[:, :CPB], junk[:], start=True,
                                 stop=True)
            for _ in range(4):
                nc.tensor.matmul(jps[:, :256], junk[:, :CPB], junk[:, :256],
                                 start=True, stop=True)

            # ---- weights first (2304B packets), then input strips ----
            wr_f32 = wstage_pool.tile([CPB, 9 * CPB], F32, tag="wstage")
            nc.sync.dma_start(wr_f32[:, :576], wr_ext[:, :576])
            nc.sync.dma_start(wr_f32[:, 576:], wr_ext[:, 576:])
            wi_f32 = wstage_pool.tile([CPB, 9 * CPB], F32, tag="wstage")
            nc.sync.dma_start(wi_f32[:, :576], wi_ext[:, :576])
            nc.sync.dma_start(wi_f32[:, 576:], wi_ext[:, 576:])

            def load_strip(img, r0, nr):
                st = stage_pool.tile([CPB, 2, 14, W], F32, tag="stage")
                src = bass.AP(
                    x2.tensor,
                    x2.offset + img * 2 * CPB * HW + r0 * W,
                    [[HW, CPB], [CPB * HW, 2], [1, nr * W]],
                )
                nc.sync.dma_start(st[:, :, :nr, :], src)
                return st

            st0a = load_strip(0, 0, 5)
            bias_sb = bias_pool.tile([CPB, 2], F32)
            nc.sync.dma_start(bias_sb[:], bias_ext[:])

            # binarized fp8 weights [ci, tap, co]; wq_s = wq_r + wq_i
            wq_r = wbin_pool.tile([CPB, 10, CPB], FP8, tag="wqr")
            wq_i = wbin_pool.tile([CPB, 10, CPB], FP8, tag="wqi")
            wq_s = wbin_pool.tile([CPB, 10, CPB], FP8, tag="wqs")
            nc.gpsimd.memset(wq_r[:, 9, :], 0.0)
            nc.gpsimd.memset(wq_i[:, 9, :], 0.0)
            wr_v = wr_f32[:].rearrange("p (t c) -> p t c", c=CPB)
            wi_v = wi_f32[:].rearrange("p (t c) -> p t c", c=CPB)
            nc.scalar.activation(wq_r[:, 0:9, :], wr_v, AF.Sign, bias=eps_pos[:], scale=1.0)
            nc.scalar.activation(wq_i[:, 0:9, :], wi_v, AF.Sign, bias=eps_pos[:], scale=1.0)

            # ---- K constants: K[co] = sum_{ci,tap} w  (KC = KA + KB) ----
            psK = psum_pool.tile([CPB, 2], F32, tag="ps", name="psK")
            for k, wq in ((0, wq_r), (1, wq_i)):
                for p in range(4):
                    rhs = bass.AP(ones2[:].tensor, ones2[:].offset,
                                  [[ones2[:].ap[0][0], CPB], [1, 2], [1, 1]])
                    nc.tensor.matmul(psK[:, k:k + 1], wq[:, 2 * p:2 * p + 2, :],
                                     rhs, start=(p == 0), stop=False,
                                     perf_mode=DRM)
                nc.tensor.matmul(psK[:, k:k + 1], wq[:, 8, :], ones2[:, 0:1],
                                 start=False, stop=True)

            # ---- persistent binarized {0,1} fp8 frames ----
            # frame row = input row + 2; frame rows 1 / 114 are the pad rows,
            # cols 0 / 113 the pad cols; pads hold 0.5 (r,i) / 1.0 (s) so
            # that x = 2*xhat - 1 (resp. xs = 2*shat - 2) is 0 there.
            xq = [xq_pool.tile([CPB, 3, FRO, RS], FP8, tag=f"xq{i}", name=f"xq{i}")
                  for i in range(IMGS)]
            for i in range(IMGS):
                eng = nc.vector if i == 0 else nc.gpsimd
                for (ks, val) in ((slice(0, 2), 0.5), (slice(2, 3), 1.0)):
                    eng.memset(xq[i][:, ks, 1:2, :], val)       # pad row top
                    eng.memset(xq[i][:, ks, 114:115, :], val)   # pad row bottom
                    eng.memset(xq[i][:, ks, 1:115, 0:1], val)   # pad col left
                    eng.memset(xq[i][:, ks, 1:115, 113:114], val)  # pad col right
                eng.memset(xq[i][:, :, 115:116, :], 0.0)  # zero-row-tap scratch

            flat = [xq[i][:].rearrange("p k r c -> p (k r c)") for i in range(IMGS)]
            KOFS = FRO * RS

            def binarize_strip(img, r0, nr, st):
                rows = slice(r0 + 2, r0 + 2 + nr)
                nc.vector.tensor_scalar(
                    xq[img][:, 0:2, rows, 1:113], st[:, :, :nr, :],
                    -EPS, None, op0=ALU.is_ge,
                )
                nc.vector.tensor_tensor(
                    xq[img][:, 2, rows, 1:113],
                    xq[img][:, 0, rows, 1:113],
                    xq[img][:, 1, rows, 1:113],
                    op=ALU.add,
                )

            binarize_strip(0, 0, 5, st0a)
            # wq_s add waits on the wq_i Sign; emit it after the first strip's
            # binarize so it can't head-of-line-block that in the vector FIFO
            nc.vector.tensor_tensor(wq_s[:], wq_r[:], wq_i[:], op=ALU.add)

            # beta1 = (br-bi)/2 + KB   (An2 = 2*Ahat + beta1)
            # beta2 = (br+bi)/2 - KA   (Bn0 = -2*Bhat + beta2)
            kvec = bias_pool.tile([CPB, 2], F32, tag="kvec")
            nc.vector.tensor_copy(kvec[:], psK[:])
            beta1 = bias_pool.tile([CPB, 1], F32, tag="b1")
            beta2 = bias_pool.tile([CPB, 1], F32, tag="b2")
            hdif = bias_pool.tile([CPB, 1], F32, tag="hd")
            hsum = bias_pool.tile([CPB, 1], F32, tag="hs")
            nc.vector.tensor_sub(hdif[:], bias_sb[:, 0:1], bias_sb[:, 1:2])
            nc.vector.tensor_add(hsum[:], bias_sb[:, 0:1], bias_sb[:, 1:2])
            nc.vector.scalar_tensor_tensor(beta1[:], hdif[:], 0.5, kvec[:, 1:2],
                                           op0=ALU.mult, op1=ALU.add)
            nc.vector.scalar_tensor_tensor(beta2[:], hsum[:], 0.5, kvec[:, 0:1],
                                           op0=ALU.mult, op1=ALU.subtract)

            st0b = load_strip(0, 5, 9)
            binarize_strip(0, 5, 9, st0b)

            # ---- conv tiles ----
            quads = {}

            def conv_tile(img, t, last_quad=False):
                xf = flat[img]
                part = [list(xf.ap)[0][0], CPB]
                pk = {}
                for ki, (kind, w3) in enumerate((("r", wq_r), ("i", wq_i),
                                                 ("s", wq_s))):
                    base = KOFS * ki + (4 * t + 2) * RS + 1
                    ps = psum_pool.tile([CPB, NT], F32, tag="ps",
                                        name=f"ps_{kind}{img}_{t}")
                    pk[kind] = ps
                    for p in range(4):
                        o0, o1 = TAP_OFF[2 * p], TAP_OFF[2 * p + 1]
                        rhs = bass.AP(
                            xf.tensor, xf.offset + base + o0,
                            [part, [o1 - o0, 2], [RS, TROWS], [1, W]],
                        )
                        nc.tensor.matmul(
                            ps[:], w3[:, 2 * p:2 * p + 2, :], rhs,
                            start=(p == 0), stop=False, perf_mode=DRM,
                        )
                    rhs8 = bass.AP(
                        xf.tensor, xf.offset + base + TAP_OFF[8],
                        [part, [RS, TROWS], [1, W]],
                    )
                    nc.tensor.matmul(ps[:], w3[:, 8, :], rhs8,
                                     start=False, stop=True)

                q, qi = t // QUAD, t % QUAD
                key = (img, q)
                if key not in quads:
                    quads[key] = out_pool.tile([CPB, 2, QUAD, NT], F32,
                                               tag="osb", name=f"osb{img}_{q}")
                ob = quads[key]

                An2 = tmp_pool.tile([CPB, NT], F32, tag="An", name=f"An{t}")
                nc.scalar.activation(An2[:], pk["r"][:], AF.Identity,
                                     bias=beta1[:], scale=2.0)
                Bn0 = tmp_pool.tile([CPB, NT], F32, tag="Bn", name=f"Bn{t}")
                nc.scalar.activation(Bn0[:], pk["i"][:], AF.Identity,
                                     bias=beta2[:], scale=-2.0)
                t5 = tmp_pool.tile([CPB, NT], F32, tag="t5", name=f"t5{t}")
                nc.vector.scalar_tensor_tensor(t5[:], pk["s"][:], 2.0, An2[:],
                                               op0=ALU.mult, op1=ALU.subtract)
                nc.gpsimd.tensor_tensor(ob[:, 0, qi], An2[:], Bn0[:], op=ALU.add)
                nc.vector.tensor_tensor(ob[:, 1, qi], t5[:], Bn0[:], op=ALU.add)

                if last_quad:
                    # final tiles: store per tile so the drain tail is short
                    dst = bass.AP(
                        out2.tensor,
                        out2.offset + img * 2 * CPB * HW + 4 * t * W,
                        [[HW, CPB], [CPB * HW, 2], [1, NT]],
                    )
                    nc.sync.dma_start(dst, ob[:, :, qi])
                elif qi == QUAD - 1:
                    dst = bass.AP(
                        out2.tensor,
                        out2.offset + img * 2 * CPB * HW + 4 * QUAD * q * W,
                        [[HW, CPB], [CPB * HW, 2], [1, QUAD * NT]],
                    )
                    nc.sync.dma_start(dst, ob[:])

            # ---- strip schedule ----
            # strip s of an image covers input rows 14s..14s+13; the first
            # tile needing it is ceil((14s - 4) / 4).  The DMA takes ~9us
            # from issue (DGE latency + queueing behind store bursts), so
            # issue it 8 tiles ahead; binarize 2 tiles ahead so the wait
            # never poisons the vector FIFO.
            first_need = [0, 3, 6, 10, 13, 17, 20, 24]
            dma_at = {}
            ts_at = {}
            pending = {}
            for img in range(IMGS):
                for s in range(8):
                    if img == 0 and s == 0:
                        continue
                    g = 28 * img + first_need[s]
                    dma_at.setdefault(max(g - 8, -1), []).append((img, s))
                    ts_at.setdefault(max(g - 3, -1), []).append((img, s))

            def do_sched(gt):
                for (img, s) in dma_at.get(gt, ()):
                    pending[(img, s)] = load_strip(img, 14 * s, 14)
                for (img, s) in ts_at.get(gt, ()):
                    binarize_strip(img, 14 * s, 14, pending.pop((img, s)))

            do_sched(-1)
            for gt in range(56):
                img, t = gt // 28, gt % 28
                conv_tile(img, t, last_quad=(gt >= 54))
                do_sched(gt)

    _split_multiwait(nc)
    return nc


def _prep(x, weight_real, weight_imag, bias):
    x = np.ascontiguousarray(np.asarray(x, dtype=np.float32))
    wr = np.asarray(weight_real, dtype=np.float32)
    wi = np.asarray(weight_imag, dtype=np.float32)
    bias = np.asarray(bias, dtype=np.float32)
    wrT = np.ascontiguousarray(wr.transpose(1, 2, 3, 0).reshape(CPB, 9 * CPB))
    wiT = np.ascontiguousarray(wi.transpose(1, 2, 3, 0).reshape(CPB, 9 * CPB))
    bias2 = np.ascontiguousarray(bias.reshape(2, CPB).T)
    return [
        {"x": x[IMGS * c:IMGS * (c + 1)], "wrT": wrT, "wiT": wiT, "bias2": bias2}
        for c in range(N_CORES)
    ]


def kernel(x, weight_real, weight_imag, bias):
    in_maps = _prep(x, weight_real, weight_imag, bias)
    nc = build_nc()
    res = run_bass_kernel_spmd(nc, in_maps, core_ids=list(range(N_CORES)))
    return np.concatenate([res.results[i]["out"] for i in range(N_CORES)], axis=0)


def run_traced(x, weight_real, weight_imag, bias, **trace_kwargs):
    """test.py entry: same as kernel() but with neuron-profile tracing."""
    in_maps = _prep(x, weight_real, weight_imag, bias)
    nc = build_nc()
    res = run_bass_kernel_spmd(
        nc, in_maps, core_ids=list(range(N_CORES)), trace=True, **trace_kwargs
    )
    out = np.concatenate([res.results[i]["out"] for i in range(N_CORES)], axis=0)
    return out, res
